# revision 6
# baseline (speedup 1.0000x reference)
"""Bass/Tile TRN2 kernel for nn_CrossAttentionLayer (B=8, NQ=64, S=4096, D=1024, H=16).

Sharding: pure data-parallel — core b computes batch element b. No collectives.

Per-core math (batch element x = queries[b] (64,1024), C = context[b] (4096,1024)):
    Q = x @ wq.T + bq ;  K = C @ wk.T + bk ;  V = C @ wv.T + bv
    per head h: P_h = softmax(Q_h K_h^T / sqrt(64)) ; O_h = P_h V_h
    out = concat_h(O_h) @ wo.T + bo

Kernel layout strategy:
  - Host pre-transposes: ctxT = C.T [D,S], qT = x.T [D,NQ], w*T = w.T [D,D]
    so every matmul contracts over the partition dim with no on-device transposes
    of big tensors.
  - K is produced transposed (KT [dout, s]) and scores are produced transposed
    (scoresT [s, nq]) so the P@V contraction (over s) needs no transpose of P.
  - Head pairs (2h, 2h+1) are packed into 128-partition tiles; scoresT for a
    pair is ONE matmul with a block-diagonal Q operand (zeros kill cross-head
    terms). P@V for a pair is ONE matmul; off-diagonal blocks are wasted MACs
    but the diagonal blocks are exactly O_h0, O_h1.
  - softmax skips the max-subtraction (logits are O(5) here, exp is safe in
    fp32) so block partial sums/PV partials are purely additive across s-blocks.
    Row sums come from an extra ones-column matmul; normalization is deferred
    to after the full PV accumulation.
  - matmul operands in bf16 (fp32 PSUM accumulation); biases added in fp32.
"""

import numpy as np
import ml_dtypes
from contextlib import ExitStack

import concourse.bass as bass
import concourse.tile as tile
from concourse import bacc, mybir
from concourse.bass_utils import run_bass_kernel_spmd
from concourse.masks import make_identity

# problem constants (hardcoded per contract)
B, NQ, S, D = 8, 64, 4096, 1024
H, HD = 16, 64
N_CORES = 8
SCALE = float(HD) ** -0.5

BF16 = mybir.dt.bfloat16
F32 = mybir.dt.float32
NPBF16 = ml_dtypes.bfloat16
AF = mybir.ActivationFunctionType

S_BLK = 1024
N_BLK = S // S_BLK
N_SUB = S_BLK // 128   # 128-row s-subtiles per block
DT = D // 128          # 128-wide dout/din tiles
NPAIR = H // 2         # head pairs (two 64-dim heads per 128-partition tile)

_PROGRAM = None


def _emit(ctx: ExitStack, tc: tile.TileContext, aps: dict, dbg: dict | None = None):
    nc = tc.nc
    ctxT, qT = aps["ctxT"], aps["qT"]
    wqT, wkT, wvT, woT = aps["wqT"], aps["wkT"], aps["wvT"], aps["woT"]
    bqr, bkr, bvr, bor = aps["bqr"], aps["bkr"], aps["bvr"], aps["bor"]
    out = aps["out"]

    const = ctx.enter_context(tc.tile_pool(name="const", bufs=1))
    wpool = ctx.enter_context(tc.tile_pool(name="wpool", bufs=1))
    ctxp = ctx.enter_context(tc.tile_pool(name="ctxp", bufs=16))
    ktp = ctx.enter_context(tc.tile_pool(name="ktp", bufs=8))
    vp = ctx.enter_context(tc.tile_pool(name="vp", bufs=8))
    expp = ctx.enter_context(tc.tile_pool(name="expp", bufs=8))
    work = ctx.enter_context(tc.tile_pool(name="work", bufs=2))
    psum_kv = ctx.enter_context(tc.tile_pool(name="psum_kv", bufs=3, space="PSUM"))
    psum_sc = ctx.enter_context(tc.tile_pool(name="psum_sc", bufs=2, space="PSUM"))
    psum_pv = ctx.enter_context(tc.tile_pool(name="psum_pv", bufs=2, space="PSUM"))
    psum_ms = ctx.enter_context(tc.tile_pool(name="psum_ms", bufs=1, space="PSUM"))

    # ---- constant / weight loads ----
    def load_w(name, dram):
        tiles = [wpool.tile([128, D], BF16, tag=f"{name}{t}", name=f"{name}{t}") for t in range(DT)]
        for t in range(DT):
            nc.sync.dma_start(tiles[t][:], dram[128 * t : 128 * (t + 1), :])
        return tiles

    wq_sb = load_w("wq", wqT)
    wk_sb = load_w("wk", wkT)
    wv_sb = load_w("wv", wvT)
    wo_sb = load_w("wo", woT)

    qt_sb = [const.tile([128, NQ], BF16, tag=f"qt{t}", name=f"qt{t}") for t in range(DT)]
    for t in range(DT):
        nc.sync.dma_start(qt_sb[t][:], qT[128 * t : 128 * (t + 1), :])

    bq_sb = const.tile([128, DT], F32, tag="bq", name="bq_sb")
    nc.sync.dma_start(bq_sb[:], bqr[:, :])
    bk_sb = const.tile([128, DT], F32, tag="bk", name="bk_sb")
    nc.sync.dma_start(bk_sb[:], bkr[:, :])
    bv_sb = const.tile([1, D], F32, tag="bv", name="bv_sb")
    nc.sync.dma_start(bv_sb[:], bvr[:, :])
    bo_sb = const.tile([1, D], F32, tag="bo", name="bo_sb")
    nc.sync.dma_start(bo_sb[:], bor[:, :])

    ident = const.tile([128, 128], F32, tag="ident", name="ident")
    make_identity(nc, ident[:])
    ones_row = const.tile([1, 128], F32, tag="ones_row", name="ones_row")
    nc.vector.memset(ones_row[:], 1.0)

    # bias row -> broadcast tiles via ones-column matmul (one-time, fp32)
    bv_bc = const.tile([128, D], F32, tag="bv_bc", name="bv_bc")
    for c in range(2):
        ps = psum_ms.tile([128, 512], F32, tag="ms", name="ms_ps")
        nc.tensor.matmul(
            ps[:], ones_row[:], bv_sb[:, 512 * c : 512 * (c + 1)], start=True, stop=True
        )
        nc.vector.tensor_copy(bv_bc[:, 512 * c : 512 * (c + 1)], ps[:])
    bo_bc = const.tile([NQ, D], F32, tag="bo_bc", name="bo_bc")
    for c in range(2):
        ps = psum_ms.tile([NQ, 512], F32, tag="ms", name="ms_ps")
        nc.tensor.matmul(
            ps[:],
            ones_row[:, 0:NQ],
            bo_sb[:, 512 * c : 512 * (c + 1)],
            start=True,
            stop=True,
        )
        nc.vector.tensor_copy(bo_bc[:, 512 * c : 512 * (c + 1)], ps[:])

    # ---- Q projection, packed block-diagonally per head pair ----
    # qt2[p] = [[Q_{2p}^T, 0], [0, Q_{2p+1}^T]]  (128 x 128, bf16)
    qt2 = []
    for t in range(DT):
        q2 = const.tile([128, 128], BF16, tag=f"qt2_{t}", name=f"qt2_{t}")
        nc.vector.memset(q2[:], 0.0)
        ps = psum_kv.tile([128, NQ], F32, tag="kv", name="kv_ps")
        for d in range(DT):
            nc.tensor.matmul(
                ps[:],
                wq_sb[d][:, 128 * t : 128 * (t + 1)],
                qt_sb[d][:],
                start=(d == 0),
                stop=(d == DT - 1),
            )
        nc.scalar.activation(
            q2[0:64, 0:64], ps[0:64, :], AF.Identity, bias=bq_sb[0:64, t : t + 1]
        )
        nc.scalar.activation(
            q2[64:128, 64:128], ps[64:128, :], AF.Identity, bias=bq_sb[64:128, t : t + 1]
        )
        qt2.append(q2)

    # ---- per-pair accumulators: [:, 0:128] = unnormalized O pair, [:,128] = rowsum
    accs = []
    for p in range(NPAIR):
        a = const.tile([128, 132], F32, tag=f"acc{p}", name=f"acc{p}")
        nc.vector.memset(a[:], 0.0)
        accs.append(a)

    # ---- main s-blocks ----
    for blk in range(N_BLK):
        s0 = blk * S_BLK
        ctx_sb = [ctxp.tile([128, S_BLK], BF16, tag="ctx", name="ctx_t") for _ in range(DT)]
        for d in range(DT):
            nc.sync.dma_start(
                ctx_sb[d][:], ctxT[128 * d : 128 * (d + 1), s0 : s0 + S_BLK]
            )

        # K projection -> KT tiles [dout 128, s S_BLK]
        kt_sb = [ktp.tile([128, S_BLK], BF16, tag="kt", name="kt_t") for _ in range(DT)]
        for t in range(DT):
            for c in range(S_BLK // 512):
                ps = psum_kv.tile([128, 512], F32, tag="kv", name="kv_ps")
                for d in range(DT):
                    nc.tensor.matmul(
                        ps[:],
                        wk_sb[d][:, 128 * t : 128 * (t + 1)],
                        ctx_sb[d][:, 512 * c : 512 * (c + 1)],
                        start=(d == 0),
                        stop=(d == DT - 1),
                    )
                nc.scalar.activation(
                    kt_sb[t][:, 512 * c : 512 * (c + 1)],
                    ps[:],
                    AF.Identity,
                    bias=bk_sb[:, t : t + 1],
                )

        # V projection -> natural V tiles [s 128, 8 pair-blocks of 144]
        # (cols 144p..144p+128 = V pair data; col 144p+128 = 1.0 for rowsums)
        v_sb = [vp.tile([128, 144 * NPAIR], BF16, tag="v", name="v_t") for _ in range(N_SUB)]
        for si in range(N_SUB):
            nc.vector.memset(v_sb[si][:], 1.0)
            for c in range(2):
                ps = psum_kv.tile([128, 512], F32, tag="kv", name="kv_ps")
                for d in range(DT):
                    nc.tensor.matmul(
                        ps[:],
                        ctx_sb[d][:, 128 * si : 128 * (si + 1)],
                        wv_sb[d][:, 512 * c : 512 * (c + 1)],
                        start=(d == 0),
                        stop=(d == DT - 1),
                    )
                for j in range(4):
                    p = 4 * c + j
                    nc.vector.tensor_add(
                        v_sb[si][:, 144 * p : 144 * p + 128],
                        ps[:, 128 * j : 128 * (j + 1)],
                        bv_bc[:, 128 * p : 128 * (p + 1)],
                    )

        if dbg is not None and blk == 0:
            nc.sync.dma_start(dbg["kt0"][:, :], kt_sb[0][:])
            nc.sync.dma_start(dbg["v0"][:, :], v_sb[0][:])

        # scoresT + exp -> expT tiles [s 128, H*64] (pair p at cols 128p..)
        exp_sb = [expp.tile([128, H * 64], BF16, tag="exp", name="exp_t") for _ in range(N_SUB)]
        for p in range(NPAIR):
            for si in range(N_SUB):
                ps = psum_sc.tile([128, 128], F32, tag="sc", name="sc_ps")
                nc.tensor.matmul(
                    ps[:],
                    kt_sb[p][:, 128 * si : 128 * (si + 1)],
                    qt2[p][:],
                    start=True,
                    stop=True,
                )
                nc.scalar.activation(
                    exp_sb[si][:, 128 * p : 128 * (p + 1)], ps[:], AF.Exp, scale=SCALE
                )

        if dbg is not None and blk == 0:
            nc.sync.dma_start(dbg["exp0"][:, :], exp_sb[0][:])

        # PV (+ embedded ones-column rowsums) per pair; accumulate into SBUF accs
        for p in range(NPAIR):
            ps = psum_pv.tile([128, 132], F32, tag="pv", name="pv_ps")
            for si in range(N_SUB):
                nc.tensor.matmul(
                    ps[:, 0:129],
                    exp_sb[si][:, 128 * p : 128 * (p + 1)],
                    v_sb[si][:, 144 * p : 144 * p + 129],
                    start=(si == 0),
                    stop=(si == N_SUB - 1),
                )
            nc.vector.tensor_add(accs[p][:, 0:129], ps[:, 0:129], accs[p][:, 0:129])

    if dbg is not None:
        nc.sync.dma_start(dbg["bv_bc"][:, :], bv_bc[:])
        nc.sync.dma_start(dbg["qt2_0"][:, :], qt2[0][:])
        nc.sync.dma_start(dbg["acc0"][:, :], accs[0][:])
        nc.sync.dma_start(dbg["acc7"][:, :], accs[7][:])

    # ---- normalize, transpose, pack O^T tiles ----
    ots = []
    for p in range(NPAIR):
        rc = work.tile([128, 1], F32, tag="recip", name="recip_t")
        nc.vector.reciprocal(rc[:], accs[p][:, 128:129])
        nm = work.tile([128, 128], F32, tag="normed", name="normed_t")
        nc.vector.tensor_scalar_mul(nm[:], accs[p][:, 0:128], rc[:])
        tp = psum_ms.tile([128, 128], F32, tag="ms", name="ms_ps")
        nc.tensor.transpose(tp[:], nm[:], ident[:])
        ot = const.tile([128, NQ], BF16, tag=f"ot{p}", name=f"ot{p}")
        nc.vector.tensor_copy(ot[0:64, :], tp[0:64, 0:64])
        nc.vector.tensor_copy(ot[64:128, :], tp[64:128, 64:128])
        ots.append(ot)
        if dbg is not None and p == 0:
            nc.sync.dma_start(dbg["ot0"][:, :], ot[:])

    # ---- output projection ----
    out_sb = const.tile([NQ, D], F32, tag="out_sb", name="out_sb")
    for c in range(2):
        ps = psum_ms.tile([NQ, 512], F32, tag="ms", name="ms_ps")
        for t in range(DT):
            nc.tensor.matmul(
                ps[:],
                ots[t][:],
                wo_sb[t][:, 512 * c : 512 * (c + 1)],
                start=(t == 0),
                stop=(t == DT - 1),
            )
        nc.vector.tensor_add(
            out_sb[:, 512 * c : 512 * (c + 1)], ps[:], bo_bc[:, 512 * c : 512 * (c + 1)]
        )
    nc.sync.dma_start(out[:, :], out_sb[:])


DBG_SHAPES = {
    "bv_bc": ([128, D], F32),
    "qt2_0": ([128, 128], BF16),
    "acc0": ([128, 132], F32),
    "acc7": ([128, 132], F32),
    "kt0": ([128, S_BLK], BF16),
    "v0": ([128, 144 * (H // 2)], BF16),
    "exp0": ([128, H * 64], BF16),
    "ot0": ([128, NQ], BF16),
}


def _build_program(debug_dumps: bool = False):
    nc = bacc.Bacc("TRN2", target_bir_lowering=False, debug=False)
    aps = {
        "ctxT": nc.dram_tensor("ctxT", [D, S], BF16, kind="ExternalInput").ap(),
        "qT": nc.dram_tensor("qT", [D, NQ], BF16, kind="ExternalInput").ap(),
        "wqT": nc.dram_tensor("wqT", [D, D], BF16, kind="ExternalInput").ap(),
        "wkT": nc.dram_tensor("wkT", [D, D], BF16, kind="ExternalInput").ap(),
        "wvT": nc.dram_tensor("wvT", [D, D], BF16, kind="ExternalInput").ap(),
        "woT": nc.dram_tensor("woT", [D, D], BF16, kind="ExternalInput").ap(),
        "bqr": nc.dram_tensor("bqr", [128, DT], F32, kind="ExternalInput").ap(),
        "bkr": nc.dram_tensor("bkr", [128, DT], F32, kind="ExternalInput").ap(),
        "bvr": nc.dram_tensor("bvr", [1, D], F32, kind="ExternalInput").ap(),
        "bor": nc.dram_tensor("bor", [1, D], F32, kind="ExternalInput").ap(),
        "out": nc.dram_tensor("out", [NQ, D], F32, kind="ExternalOutput").ap(),
    }
    dbg = None
    if debug_dumps:
        dbg = {
            k: nc.dram_tensor(f"dbg_{k}", shp, dt, kind="ExternalOutput").ap()
            for k, (shp, dt) in DBG_SHAPES.items()
        }
    with tile.TileContext(nc) as tc:
        with ExitStack() as stack:
            _emit(stack, tc, aps, dbg)
    nc.compile()
    return nc


def _get_program():
    global _PROGRAM
    if _PROGRAM is None:
        _PROGRAM = _build_program()
    return _PROGRAM


def make_in_maps(inputs: dict) -> list[dict]:
    q = np.asarray(inputs["queries"], np.float32)
    ctxf = np.asarray(inputs["context"], np.float32)
    shared = {}
    for wname, key in (("wq", "wqT"), ("wk", "wkT"), ("wv", "wvT"), ("wo", "woT")):
        w = np.asarray(inputs[wname], np.float32)
        shared[key] = w.T.astype(NPBF16, order="C")
    shared["bqr"] = np.ascontiguousarray(
        np.asarray(inputs["bq"], np.float32).reshape(DT, 128).T
    )
    shared["bkr"] = np.ascontiguousarray(
        np.asarray(inputs["bk"], np.float32).reshape(DT, 128).T
    )
    shared["bvr"] = np.asarray(inputs["bv"], np.float32).reshape(1, D).copy()
    shared["bor"] = np.asarray(inputs["bo"], np.float32).reshape(1, D).copy()

    in_maps = []
    for b in range(B):
        m = dict(shared)
        m["ctxT"] = ctxf[b].T.astype(NPBF16, order="C")
        m["qT"] = q[b].T.astype(NPBF16, order="C")
        in_maps.append(m)
    return in_maps


def kernel(**inputs) -> np.ndarray:
    nc = _get_program()
    in_maps = make_in_maps(inputs)
    res = run_bass_kernel_spmd(nc, in_maps, core_ids=list(range(N_CORES)))
    return np.stack([res.results[b]["out"] for b in range(B)]).astype(np.float32)


# revision 8
# speedup vs baseline: 12.3092x; 12.3092x over previous
"""Bass/Tile TRN2 kernel for nn_CrossAttentionLayer (B=8, NQ=64, S=4096, D=1024, H=16).

Sharding: pure data-parallel — core b computes batch element b. No collectives.

Per-core math (batch element x = queries[b] (64,1024), C = context[b] (4096,1024)):
    Q = x @ wq.T + bq ;  K = C @ wk.T + bk ;  V = C @ wv.T + bv
    per head h: P_h = softmax(Q_h K_h^T / sqrt(64)) ; O_h = P_h V_h
    out = concat_h(O_h) @ wo.T + bo

Kernel layout strategy:
  - Host pre-transposes: ctxT = C.T [D,S], qT = x.T [D,NQ], w*T = w.T [D,D]
    so every matmul contracts over the partition dim with no on-device transposes
    of big tensors.
  - K is produced transposed (KT [dout, s]) and scores are produced transposed
    (scoresT [s, nq]) so the P@V contraction (over s) needs no transpose of P.
  - Head pairs (2h, 2h+1) are packed into 128-partition tiles; scoresT for a
    pair is ONE matmul with a block-diagonal Q operand (zeros kill cross-head
    terms). P@V for a pair is ONE matmul; off-diagonal blocks are wasted MACs
    but the diagonal blocks are exactly O_h0, O_h1.
  - softmax skips the max-subtraction (logits are O(5) here, exp is safe in
    fp32) so block partial sums/PV partials are purely additive across s-blocks.
    Row sums come from an extra ones-column matmul; normalization is deferred
    to after the full PV accumulation.
  - matmul operands in bf16 (fp32 PSUM accumulation); biases added in fp32.
"""

import numpy as np
import ml_dtypes
from contextlib import ExitStack

import concourse.bass as bass
import concourse.tile as tile
from concourse import bacc, mybir
from concourse.bass_utils import run_bass_kernel_spmd
from concourse.masks import make_identity

# problem constants (hardcoded per contract)
B, NQ, S, D = 8, 64, 4096, 1024
H, HD = 16, 64
N_CORES = 8
SCALE = float(HD) ** -0.5

BF16 = mybir.dt.bfloat16
F32 = mybir.dt.float32
NPBF16 = ml_dtypes.bfloat16
AF = mybir.ActivationFunctionType

S_BLK = 1024
N_BLK = S // S_BLK
N_SUB = S_BLK // 128   # 128-row s-subtiles per block
DT = D // 128          # 128-wide dout/din tiles
NPAIR = H // 2         # head pairs (two 64-dim heads per 128-partition tile)

_PROGRAM = None


def _emit(ctx: ExitStack, tc: tile.TileContext, aps: dict, dbg: dict | None = None, repeat: int = 1):
    nc = tc.nc
    ctxT, qT = aps["ctxT"], aps["qT"]
    wqT, wkT, wvT, woT = aps["wqT"], aps["wkT"], aps["wvT"], aps["woT"]
    bqr, bkr, bvr, bor = aps["bqr"], aps["bkr"], aps["bvr"], aps["bor"]
    out = aps["out"]

    const = ctx.enter_context(tc.tile_pool(name="const", bufs=1))
    wpool = ctx.enter_context(tc.tile_pool(name="wpool", bufs=1))
    ctxp = ctx.enter_context(tc.tile_pool(name="ctxp", bufs=16))
    ktp = ctx.enter_context(tc.tile_pool(name="ktp", bufs=8))
    vp = ctx.enter_context(tc.tile_pool(name="vp", bufs=8))
    expp = ctx.enter_context(tc.tile_pool(name="expp", bufs=8))
    work = ctx.enter_context(tc.tile_pool(name="work", bufs=2))
    psum_kv = ctx.enter_context(tc.tile_pool(name="psum_kv", bufs=3, space="PSUM"))
    psum_sc = ctx.enter_context(tc.tile_pool(name="psum_sc", bufs=2, space="PSUM"))
    psum_pv = ctx.enter_context(tc.tile_pool(name="psum_pv", bufs=2, space="PSUM"))
    psum_ms = ctx.enter_context(tc.tile_pool(name="psum_ms", bufs=1, space="PSUM"))

    # ---- constant / weight loads ----
    def load_w(name, dram):
        tiles = [wpool.tile([128, D], BF16, tag=f"{name}{t}", name=f"{name}{t}") for t in range(DT)]
        for t in range(DT):
            nc.sync.dma_start(tiles[t][:], dram[128 * t : 128 * (t + 1), :])
        return tiles

    wq_sb = load_w("wq", wqT)
    wk_sb = load_w("wk", wkT)
    wv_sb = load_w("wv", wvT)
    wo_sb = load_w("wo", woT)

    qt_sb = [const.tile([128, NQ], BF16, tag=f"qt{t}", name=f"qt{t}") for t in range(DT)]
    for t in range(DT):
        nc.sync.dma_start(qt_sb[t][:], qT[128 * t : 128 * (t + 1), :])

    bq_sb = const.tile([128, DT], F32, tag="bq", name="bq_sb")
    nc.sync.dma_start(bq_sb[:], bqr[:, :])
    bk_sb = const.tile([128, DT], F32, tag="bk", name="bk_sb")
    nc.sync.dma_start(bk_sb[:], bkr[:, :])
    bv_sb = const.tile([1, D], F32, tag="bv", name="bv_sb")
    nc.sync.dma_start(bv_sb[:], bvr[:, :])
    bo_sb = const.tile([1, D], F32, tag="bo", name="bo_sb")
    nc.sync.dma_start(bo_sb[:], bor[:, :])

    ident = const.tile([128, 128], F32, tag="ident", name="ident")
    make_identity(nc, ident[:])
    ones_row = const.tile([1, 128], F32, tag="ones_row", name="ones_row")
    nc.vector.memset(ones_row[:], 1.0)

    # bias row -> broadcast tiles via ones-column matmul (one-time, fp32)
    bv_bc = const.tile([128, D], F32, tag="bv_bc", name="bv_bc")
    for c in range(2):
        ps = psum_ms.tile([128, 512], F32, tag="ms", name="ms_ps")
        nc.tensor.matmul(
            ps[:], ones_row[:], bv_sb[:, 512 * c : 512 * (c + 1)], start=True, stop=True
        )
        nc.vector.tensor_copy(bv_bc[:, 512 * c : 512 * (c + 1)], ps[:])
    bo_bc = const.tile([NQ, D], F32, tag="bo_bc", name="bo_bc")
    for c in range(2):
        ps = psum_ms.tile([NQ, 512], F32, tag="ms", name="ms_ps")
        nc.tensor.matmul(
            ps[:],
            ones_row[:, 0:NQ],
            bo_sb[:, 512 * c : 512 * (c + 1)],
            start=True,
            stop=True,
        )
        nc.vector.tensor_copy(bo_bc[:, 512 * c : 512 * (c + 1)], ps[:])

    # ---- Q projection, packed block-diagonally per head pair ----
    # qt2[p] = [[Q_{2p}^T, 0], [0, Q_{2p+1}^T]]  (128 x 128, bf16)
    qt2 = []
    for t in range(DT):
        q2 = const.tile([128, 128], BF16, tag=f"qt2_{t}", name=f"qt2_{t}")
        nc.vector.memset(q2[:], 0.0)
        ps = psum_kv.tile([128, NQ], F32, tag="kv", name="kv_ps")
        for d in range(DT):
            nc.tensor.matmul(
                ps[:],
                wq_sb[d][:, 128 * t : 128 * (t + 1)],
                qt_sb[d][:],
                start=(d == 0),
                stop=(d == DT - 1),
            )
        nc.scalar.activation(
            q2[0:64, 0:64], ps[0:64, :], AF.Identity, bias=bq_sb[0:64, t : t + 1]
        )
        nc.scalar.activation(
            q2[64:128, 64:128], ps[64:128, :], AF.Identity, bias=bq_sb[64:128, t : t + 1]
        )
        qt2.append(q2)

    # ---- per-pair accumulators: [:, 0:128] = unnormalized O pair, [:,128] = rowsum
    accs = []
    for p in range(NPAIR):
        a = const.tile([128, 132], F32, tag=f"acc{p}", name=f"acc{p}")
        accs.append(a)

    for _rep in range(repeat):
        _emit_body(tc, aps, dbg if _rep == 0 else None, locals())


def _emit_body(tc: tile.TileContext, aps: dict, dbg: dict | None, env: dict):
    nc = tc.nc
    ctxT, out = aps["ctxT"], aps["out"]
    ctxp, ktp, vp, expp, work = env["ctxp"], env["ktp"], env["vp"], env["expp"], env["work"]
    psum_kv, psum_sc, psum_pv, psum_ms = env["psum_kv"], env["psum_sc"], env["psum_pv"], env["psum_ms"]
    wk_sb, wv_sb, wo_sb = env["wk_sb"], env["wv_sb"], env["wo_sb"]
    bk_sb, bv_bc, bo_bc = env["bk_sb"], env["bv_bc"], env["bo_bc"]
    ident, qt2, accs = env["ident"], env["qt2"], env["accs"]
    const = env["const"]

    for p in range(NPAIR):
        nc.vector.memset(accs[p][:], 0.0)

    # ---- main s-blocks ----
    for blk in range(N_BLK):
        s0 = blk * S_BLK
        ctx_sb = [ctxp.tile([128, S_BLK], BF16, tag="ctx", name="ctx_t") for _ in range(DT)]
        for d in range(DT):
            nc.sync.dma_start(
                ctx_sb[d][:], ctxT[128 * d : 128 * (d + 1), s0 : s0 + S_BLK]
            )

        # K projection -> KT tiles [dout 128, s S_BLK]
        kt_sb = [ktp.tile([128, S_BLK], BF16, tag="kt", name="kt_t") for _ in range(DT)]
        for t in range(DT):
            for c in range(S_BLK // 512):
                ps = psum_kv.tile([128, 512], F32, tag="kv", name="kv_ps")
                for d in range(DT):
                    nc.tensor.matmul(
                        ps[:],
                        wk_sb[d][:, 128 * t : 128 * (t + 1)],
                        ctx_sb[d][:, 512 * c : 512 * (c + 1)],
                        start=(d == 0),
                        stop=(d == DT - 1),
                    )
                nc.scalar.activation(
                    kt_sb[t][:, 512 * c : 512 * (c + 1)],
                    ps[:],
                    AF.Identity,
                    bias=bk_sb[:, t : t + 1],
                )

        # V projection -> natural V tiles [s 128, 8 pair-blocks of 144]
        # (cols 144p..144p+128 = V pair data; col 144p+128 = 1.0 for rowsums)
        v_sb = [vp.tile([128, 144 * NPAIR], BF16, tag="v", name="v_t") for _ in range(N_SUB)]
        for si in range(N_SUB):
            nc.vector.memset(v_sb[si][:], 1.0)
            for c in range(2):
                ps = psum_kv.tile([128, 512], F32, tag="kv", name="kv_ps")
                for d in range(DT):
                    nc.tensor.matmul(
                        ps[:],
                        ctx_sb[d][:, 128 * si : 128 * (si + 1)],
                        wv_sb[d][:, 512 * c : 512 * (c + 1)],
                        start=(d == 0),
                        stop=(d == DT - 1),
                    )
                for j in range(4):
                    p = 4 * c + j
                    nc.vector.tensor_add(
                        v_sb[si][:, 144 * p : 144 * p + 128],
                        ps[:, 128 * j : 128 * (j + 1)],
                        bv_bc[:, 128 * p : 128 * (p + 1)],
                    )

        if dbg is not None and blk == 0:
            nc.sync.dma_start(dbg["kt0"][:, :], kt_sb[0][:])
            nc.sync.dma_start(dbg["v0"][:, :], v_sb[0][:])

        # scoresT + exp -> expT tiles [s 128, H*64] (pair p at cols 128p..)
        exp_sb = [expp.tile([128, H * 64], BF16, tag="exp", name="exp_t") for _ in range(N_SUB)]
        for p in range(NPAIR):
            for si in range(N_SUB):
                ps = psum_sc.tile([128, 128], F32, tag="sc", name="sc_ps")
                nc.tensor.matmul(
                    ps[:],
                    kt_sb[p][:, 128 * si : 128 * (si + 1)],
                    qt2[p][:],
                    start=True,
                    stop=True,
                )
                nc.scalar.activation(
                    exp_sb[si][:, 128 * p : 128 * (p + 1)], ps[:], AF.Exp, scale=SCALE
                )

        if dbg is not None and blk == 0:
            nc.sync.dma_start(dbg["exp0"][:, :], exp_sb[0][:])

        # PV (+ embedded ones-column rowsums) per pair; accumulate into SBUF accs
        for p in range(NPAIR):
            ps = psum_pv.tile([128, 132], F32, tag="pv", name="pv_ps")
            for si in range(N_SUB):
                nc.tensor.matmul(
                    ps[:, 0:129],
                    exp_sb[si][:, 128 * p : 128 * (p + 1)],
                    v_sb[si][:, 144 * p : 144 * p + 129],
                    start=(si == 0),
                    stop=(si == N_SUB - 1),
                )
            nc.vector.tensor_add(accs[p][:, 0:129], ps[:, 0:129], accs[p][:, 0:129])

    if dbg is not None:
        nc.sync.dma_start(dbg["bv_bc"][:, :], bv_bc[:])
        nc.sync.dma_start(dbg["qt2_0"][:, :], qt2[0][:])
        nc.sync.dma_start(dbg["acc0"][:, :], accs[0][:])
        nc.sync.dma_start(dbg["acc7"][:, :], accs[7][:])

    # ---- normalize, transpose, pack O^T tiles ----
    ots = []
    for p in range(NPAIR):
        rc = work.tile([128, 1], F32, tag="recip", name="recip_t")
        nc.vector.reciprocal(rc[:], accs[p][:, 128:129])
        nm = work.tile([128, 128], F32, tag="normed", name="normed_t")
        nc.vector.tensor_scalar_mul(nm[:], accs[p][:, 0:128], rc[:])
        tp = psum_ms.tile([128, 128], F32, tag="ms", name="ms_ps")
        nc.tensor.transpose(tp[:], nm[:], ident[:])
        ot = const.tile([128, NQ], BF16, tag=f"ot{p}", name=f"ot{p}")
        nc.vector.tensor_copy(ot[0:64, :], tp[0:64, 0:64])
        nc.vector.tensor_copy(ot[64:128, :], tp[64:128, 64:128])
        ots.append(ot)
        if dbg is not None and p == 0:
            nc.sync.dma_start(dbg["ot0"][:, :], ot[:])

    # ---- output projection ----
    out_sb = const.tile([NQ, D], F32, tag="out_sb", name="out_sb")
    for c in range(2):
        ps = psum_ms.tile([NQ, 512], F32, tag="ms", name="ms_ps")
        for t in range(DT):
            nc.tensor.matmul(
                ps[:],
                ots[t][:],
                wo_sb[t][:, 512 * c : 512 * (c + 1)],
                start=(t == 0),
                stop=(t == DT - 1),
            )
        nc.vector.tensor_add(
            out_sb[:, 512 * c : 512 * (c + 1)], ps[:], bo_bc[:, 512 * c : 512 * (c + 1)]
        )
    nc.sync.dma_start(out[:, :], out_sb[:])


DBG_SHAPES = {
    "bv_bc": ([128, D], F32),
    "qt2_0": ([128, 128], BF16),
    "acc0": ([128, 132], F32),
    "acc7": ([128, 132], F32),
    "kt0": ([128, S_BLK], BF16),
    "v0": ([128, 144 * (H // 2)], BF16),
    "exp0": ([128, H * 64], BF16),
    "ot0": ([128, NQ], BF16),
}


def _build_program(debug_dumps: bool = False, repeat: int = 1):
    nc = bacc.Bacc("TRN2", target_bir_lowering=False, debug=False)
    aps = {
        "ctxT": nc.dram_tensor("ctxT", [D, S], BF16, kind="ExternalInput").ap(),
        "qT": nc.dram_tensor("qT", [D, NQ], BF16, kind="ExternalInput").ap(),
        "wqT": nc.dram_tensor("wqT", [D, D], BF16, kind="ExternalInput").ap(),
        "wkT": nc.dram_tensor("wkT", [D, D], BF16, kind="ExternalInput").ap(),
        "wvT": nc.dram_tensor("wvT", [D, D], BF16, kind="ExternalInput").ap(),
        "woT": nc.dram_tensor("woT", [D, D], BF16, kind="ExternalInput").ap(),
        "bqr": nc.dram_tensor("bqr", [128, DT], F32, kind="ExternalInput").ap(),
        "bkr": nc.dram_tensor("bkr", [128, DT], F32, kind="ExternalInput").ap(),
        "bvr": nc.dram_tensor("bvr", [1, D], F32, kind="ExternalInput").ap(),
        "bor": nc.dram_tensor("bor", [1, D], F32, kind="ExternalInput").ap(),
        "out": nc.dram_tensor("out", [NQ, D], F32, kind="ExternalOutput").ap(),
    }
    dbg = None
    if debug_dumps:
        dbg = {
            k: nc.dram_tensor(f"dbg_{k}", shp, dt, kind="ExternalOutput").ap()
            for k, (shp, dt) in DBG_SHAPES.items()
        }
    with tile.TileContext(nc) as tc:
        with ExitStack() as stack:
            _emit(stack, tc, aps, dbg, repeat=repeat)
    nc.compile()
    return nc


def _get_program():
    global _PROGRAM
    if _PROGRAM is None:
        _PROGRAM = _build_program()
    return _PROGRAM


def make_in_maps(inputs: dict) -> list[dict]:
    q = np.asarray(inputs["queries"], np.float32)
    ctxf = np.asarray(inputs["context"], np.float32)
    shared = {}
    for wname, key in (("wq", "wqT"), ("wk", "wkT"), ("wv", "wvT"), ("wo", "woT")):
        w = np.asarray(inputs[wname], np.float32)
        shared[key] = w.T.astype(NPBF16, order="C")
    shared["bqr"] = np.ascontiguousarray(
        np.asarray(inputs["bq"], np.float32).reshape(DT, 128).T
    )
    shared["bkr"] = np.ascontiguousarray(
        np.asarray(inputs["bk"], np.float32).reshape(DT, 128).T
    )
    shared["bvr"] = np.asarray(inputs["bv"], np.float32).reshape(1, D).copy()
    shared["bor"] = np.asarray(inputs["bo"], np.float32).reshape(1, D).copy()

    in_maps = []
    for b in range(B):
        m = dict(shared)
        m["ctxT"] = ctxf[b].T.astype(NPBF16, order="C")
        m["qT"] = q[b].T.astype(NPBF16, order="C")
        in_maps.append(m)
    return in_maps


def kernel(**inputs) -> np.ndarray:
    nc = _get_program()
    in_maps = make_in_maps(inputs)
    res = run_bass_kernel_spmd(nc, in_maps, core_ids=list(range(N_CORES)))
    return np.stack([res.results[b]["out"] for b in range(B)]).astype(np.float32)
